# revision 27
# baseline (speedup 1.0000x reference)
"""Trainium2 kernel for nn_PerfeCT (retrieval_knn set-membership).

Semantics (matches the reference as executed in this environment):
  key(q) = (h*15000 + r)*15000 + t   computed in the input integer dtype
  (int32 inputs -> int32 wraparound; int64 inputs -> exact 42-bit keys)
  out[i] = 10 * (member(key_i) - 0.5)  as float32, member in {0, 1}.

Distribution strategy (the sharding hint's "replicate the sorted key
table and data-parallel shard the queries" alternative):
  * The host builds a bucketed key table: bucket = high bits of the key,
    tag = the remaining low bits; (bucket, tag) <-> key bijectively, so
    membership of a key == "tag appears in its bucket's row" (exact).
  * The table is sharded by bucket range across the 8 cores; each query
    is routed (on host) to the core owning its bucket. Within a core the
    bucket range is split into two halves (dma_gather indices are int16,
    so one gather stream addresses at most 32768 rows); queries are
    grouped by half.
  * Each core: chunked dma_gather pulls the 256B bucket row for each of
    its queries. Descriptor generation is the bottleneck (~8ns/descriptor
    per Q7 cpu pair), so chunks are spread round-robin over all 4 SWDGE
    queues (= all 4 cpu pairs) and sized so transfers/compares pipeline
    behind descgen. The vector engine compares gathered rows against the
    query tags (is_equal + reduce-max), and an affine op maps the hit
    bit to +/-5.0.
  * Host scatters the per-core results back to the original query order.
"""

import math

import numpy as np

import concourse.bass as bass  # noqa: F401
import concourse.mybir as mybir
from concourse import bacc
from concourse.bass_utils import run_bass_kernel_spmd
from concourse.library_config import mlp

N_ENT = 15000
N_CORES = 8
LOGB = 19            # total buckets = 2**LOGB; 2 gather streams per core
HALF = 32768         # rows per gather stream (int16 index limit)
P = 128

LAST_RESULTS = None  # BassKernelResults of the most recent kernel() call

N_QUEUES = 4         # SWDGE queues; each runs on its own Q7 cpu pair
CHUNK_BLOCKS = 8     # 128-query blocks per dma_gather chunk


def _plan_chunks(G_half: int):
    """Chunks of (g0, cb, queue, half) covering 2*G_half blocks.

    Each chunk stays within one table half. Chunks go round-robin over
    the 4 SWDGE queues (cpu pairs run in parallel; a pair's own chunks
    are serial). A chunk's DMA-completion sem lags its descgen end by
    roughly 0.7us per block (ring drain), so chunk sizes DESCEND toward
    the end of the stream to keep the tail short.
    """
    def split4(n):
        """n split into 4 near-equal parts."""
        b = n // N_QUEUES
        r = n - b * N_QUEUES
        return [b + (1 if i < r else 0) for i in range(N_QUEUES)]

    rounds = []  # each round: list of (cb, h) per queue, cb may be 0
    if G_half >= 2 * N_QUEUES + 8:
        h0 = split4(G_half)
        h1 = split4(G_half)
        # half0: a small first round (early first sem for the vector
        # engine) + the bulk; half1: bulk + descending 3,1 tail
        rounds.append([(min(4, c - 1), 0) for c in h0])
        rounds.append([(c - min(4, c - 1), 0) for c in h0])
        rounds.append([(c - 4, 1) for c in h1])
        rounds.append([(3, 1) for _ in h1])
        rounds.append([(1, 1) for _ in h1])
    else:  # tiny problem fallback: one chunk per queue per half
        rounds.append([(c, 0) for c in split4(G_half)])
        rounds.append([(c, 1) for c in split4(G_half)])

    chunks = []
    g0 = 0
    for rnd in rounds:
        for q, (cb, h) in enumerate(rnd):
            if cb <= 0:
                continue
            chunks.append((g0, cb, q, h))
            g0 += cb
    assert g0 == 2 * G_half, (g0, G_half)
    return chunks


def _build_nc(G_half: int, CAP: int, CAPC: int, tag_dt: "mybir.dt"):
    """Device program: probe 2*G_half*128 queries against two [HALF, CAP]
    tag-table halves.

    CAP is the gathered row length (dma_gather needs 256B multiples);
    CAPC <= CAP is the occupied prefix actually compared.
    """
    nc = bacc.Bacc(
        "TRN2", target_bir_lowering=False, debug=False, num_swdge_queues=N_QUEUES
    )
    G = 2 * G_half
    Qc = G * P
    chunks = _plan_chunks(G_half)
    max_cb = max(cb for _, cb, _, _ in chunks)

    t0 = nc.dram_tensor("t0", [HALF, CAP], tag_dt, kind="ExternalInput")
    t1 = nc.dram_tensor("t1", [HALF, CAP], tag_dt, kind="ExternalInput")
    idxw_d = nc.dram_tensor("idxw", [P, Qc // 16], mybir.dt.int16, kind="ExternalInput")
    qtag_d = nc.dram_tensor("qtag", [P, G], tag_dt, kind="ExternalInput")
    out_d = nc.dram_tensor("hit", [P, G], mybir.dt.float32, kind="ExternalOutput")

    with (
        nc.Block() as block,
        nc.sbuf_tensor("iw", [P, Qc // 16], mybir.dt.int16) as iw,
        nc.sbuf_tensor("tagt", [P, G], tag_dt) as tagt,
        nc.sbuf_tensor("gt", [P, G, CAP], tag_dt) as gt,
        nc.sbuf_tensor("eq", [P, max_cb, CAPC], mybir.dt.bfloat16) as eq,
        nc.sbuf_tensor("m", [P, G], mybir.dt.bfloat16) as m,
        nc.sbuf_tensor("res", [P, G], mybir.dt.float32) as res,
        nc.semaphore("s_iw") as s_iw,
        nc.semaphore("s_tag") as s_tag,
        nc.semaphore("s_g0") as s_g0,
        nc.semaphore("s_g1") as s_g1,
        nc.semaphore("s_g2") as s_g2,
        nc.semaphore("s_g3") as s_g3,
        nc.semaphore("s_v") as s_v,
        nc.semaphore("s_out") as s_out,
    ):
        s_gs = [s_g0, s_g1, s_g2, s_g3]
        tables = [t0, t1]

        @block.gpsimd
        def _(g):
            g.load_library(mlp)
            g.wait_ge(s_iw, 16)  # idx array resident (tags only gate vector)
            for g0, cb, q, h in chunks:
                cq = cb * P
                g.dma_gather(
                    gt[:, g0 : g0 + cb, :], tables[h].ap(),
                    iw[:, g0 * (P // 16) : (g0 + cb) * (P // 16)],
                    cq, cq, CAP, single_packet=False,
                    queue_num=q,
                ).then_inc(s_gs[q], 16)

        @block.vector
        def _(v):
            seen = [0] * N_QUEUES
            for k, (g0, cb, q, h) in enumerate(chunks):
                seen[q] += 1
                v.wait_ge(s_gs[q], 16 * seen[q])
                if k == 0:
                    v.wait_ge(s_tag, 16)
                v.tensor_tensor(
                    out=eq[:, :cb, :],
                    in0=gt[:, g0 : g0 + cb, :CAPC],
                    in1=tagt[:, g0 : g0 + cb].to_broadcast([P, cb, CAPC]),
                    op=mybir.AluOpType.is_equal,
                )
                v.tensor_reduce(
                    out=m[:, g0 : g0 + cb], in_=eq[:, :cb, :],
                    axis=mybir.AxisListType.X, op=mybir.AluOpType.max,
                )
            v.tensor_scalar(
                out=res[:], in0=m[:], scalar1=10.0, scalar2=-5.0,
                op0=mybir.AluOpType.mult, op1=mybir.AluOpType.add,
            ).then_inc(s_v, 1)

        @block.sync
        def _(sy):
            sy.dma_start(iw[:], idxw_d.ap()).then_inc(s_iw, 16)
            sy.dma_start(tagt[:], qtag_d.ap()).then_inc(s_tag, 16)
            sy.wait_ge(s_v, 1)
            sy.dma_start(out_d.ap(), res[:]).then_inc(s_out, 16)
            sy.wait_ge(s_out, 16)

    nc.compile()
    return nc


def _ensure_trace_hook():
    """If BASS_TRACE is set but this image's antenv lacks axon_hooks,
    bass_utils would crash on import; synthesize the module (real ctypes
    hook when available, else a None hook so tracing degrades gracefully)."""
    import sys
    import types

    try:
        import antenv.axon_hooks  # noqa: F401
        return
    except ImportError:
        pass
    hook = None
    try:
        from trn_agent_boot.trn_boot import _ntff_profile_via_ctypes

        hook = _ntff_profile_via_ctypes("/opt/axon/libaxon_pjrt.so")
    except Exception:
        hook = None
    mod = types.ModuleType("antenv.axon_hooks")
    mod.get_axon_ntff_profile_hook = lambda: hook
    mod.set_axon_ntff_profile_hook = lambda h: None
    sys.modules["antenv.axon_hooks"] = mod


def _keys(h, r, t, int64_mode):
    """Replicates the reference's key computation."""
    if int64_mode:
        h = h.astype(np.int64)
        return (h * 15000 + r.astype(np.int64)) * 15000 + t.astype(np.int64)
    # int32 path: jax with x64 disabled wraps in int32; compute in uint32
    # (same bit pattern, well-defined wraparound).
    h = h.astype(np.uint32)
    return (h * np.uint32(15000) + r.astype(np.uint32)) * np.uint32(15000) + t.astype(
        np.uint32
    )


def kernel(heads, rels, tails, data) -> np.ndarray:
    heads = np.ascontiguousarray(heads)
    rels = np.ascontiguousarray(rels)
    tails = np.ascontiguousarray(tails)
    data = np.ascontiguousarray(data)
    Q = heads.shape[0]

    int64_mode = bool(heads.dtype == np.int64 or data.dtype == np.int64)
    keybits = 42 if int64_mode else 32
    shift = keybits - LOGB
    tag_mask = (1 << shift) - 1
    tag_np = np.int32 if shift > 15 else np.int16
    tag_dt = mybir.dt.int32 if shift > 15 else mybir.dt.int16
    # dma_gather rows must be a multiple of 256 bytes
    cap_quantum = 256 // np.dtype(tag_np).itemsize

    dk = _keys(data[0], data[1], data[2], int64_mode)
    qk = _keys(heads, rels, tails, int64_mode)

    # --- table build (host): sort keys; high bits = bucket -> contiguous runs
    B = 1 << LOGB
    nbl_core = B // N_CORES            # buckets per core (2 halves of HALF)
    assert nbl_core == 2 * HALF
    ds = np.sort(dk)
    db = (ds >> shift).astype(np.int64)
    dtag = (ds & np.array(tag_mask, dtype=ds.dtype)).astype(tag_np)
    counts = np.bincount(db, minlength=B)
    CAPC = max(8, int(math.ceil(counts.max() / 8)) * 8)          # compared slots
    CAP = max(cap_quantum, int(math.ceil(CAPC / cap_quantum)) * cap_quantum)
    starts = np.zeros(B, dtype=np.int64)
    np.cumsum(counts[:-1], out=starts[1:])
    slot = np.arange(ds.shape[0], dtype=np.int64) - starts[db]
    table = np.full((B, CAP), -1, dtype=tag_np)
    table[db, slot] = dtag

    # --- query routing (host)
    qb = (qk >> shift).astype(np.int64)
    qtag = (qk & np.array(tag_mask, dtype=qk.dtype)).astype(tag_np)
    qcore = qb >> (LOGB - 3)
    qhalf = (qb >> 15) & 1
    qlocal = (qb & (HALF - 1)).astype(np.int16)
    sels = [
        np.nonzero((qcore == c) & (qhalf == h))[0]
        for c in range(N_CORES)
        for h in (0, 1)
    ]  # index: 2*c + h
    G_half = max(1, int(math.ceil(max(len(s) for s in sels) / P)))
    G = 2 * G_half
    Qc = G * P
    chunks = _plan_chunks(G_half)

    in_maps = []
    for c in range(N_CORES):
        idx_flat = np.zeros(Qc, dtype=np.int16)      # padding gathers row 0 (harmless)
        tag_t = np.full((G, P), -2, dtype=tag_np)    # padding never matches
        for h in (0, 1):
            s = sels[2 * c + h]
            o = h * G_half * P
            idx_flat[o : o + len(s)] = qlocal[s]
            tag_t.ravel()[o : o + len(s)] = qtag[s]
            # NOTE: padding must stay 0 (gather row 0). Trimming pads with
            # -1 (the ucode drops trailing negatives) hangs the kernel on
            # hardware: the chunk's DMA-completion sem then undershoots
            # the +16 the waiters expect.
        idx_w = np.tile(idx_flat.reshape(-1, 16).T, (8, 1))  # [128, Qc//16]
        base = c * nbl_core
        in_maps.append(
            {
                "t0": table[base : base + HALF],
                "t1": table[base + HALF : base + 2 * HALF],
                "idxw": np.ascontiguousarray(idx_w),
                "qtag": np.ascontiguousarray(tag_t.T),
            }
        )

    _ensure_trace_hook()
    nc = _build_nc(G_half, CAP, CAPC, tag_dt)
    # trace_cores=all: profiling a strict subset of executing cores crashes
    # the axon NRT profile path; all-cores tracing is stable.
    r = run_bass_kernel_spmd(
        nc, in_maps, core_ids=list(range(N_CORES)),
        trace_cores=list(range(N_CORES)),
    )
    global LAST_RESULTS
    LAST_RESULTS = r

    out = np.full(Q, -5.0, dtype=np.float32)
    for c in range(N_CORES):
        res = r.results[c]["hit"]  # [P, G]
        flat = res.T.ravel()
        for h in (0, 1):
            s = sels[2 * c + h]
            o = h * G_half * P
            out[s] = flat[o : o + len(s)]
    return out


# revision 28
# speedup vs baseline: 1.0123x; 1.0123x over previous
"""Trainium2 kernel for nn_PerfeCT (retrieval_knn set-membership).

Semantics (matches the reference as executed in this environment):
  key(q) = (h*15000 + r)*15000 + t   computed in the input integer dtype
  (int32 inputs -> int32 wraparound; int64 inputs -> exact 42-bit keys)
  out[i] = 10 * (member(key_i) - 0.5)  as float32, member in {0, 1}.

Distribution strategy (the sharding hint's "replicate the sorted key
table and data-parallel shard the queries" alternative):
  * The host builds a bucketed key table: bucket = high bits of the key,
    tag = the remaining low bits; (bucket, tag) <-> key bijectively, so
    membership of a key == "tag appears in its bucket's row" (exact).
  * The table is sharded by bucket range across the 8 cores; each query
    is routed (on host) to the core owning its bucket. Within a core the
    bucket range is split into two halves (dma_gather indices are int16,
    so one gather stream addresses at most 32768 rows); queries are
    grouped by half.
  * Each core: chunked dma_gather pulls the 256B bucket row for each of
    its queries. Descriptor generation is the bottleneck (~8ns/descriptor
    per Q7 cpu pair), so chunks are spread round-robin over all 4 SWDGE
    queues (= all 4 cpu pairs) and sized so transfers/compares pipeline
    behind descgen. The vector engine compares gathered rows against the
    query tags (is_equal + reduce-max), and an affine op maps the hit
    bit to +/-5.0.
  * Host scatters the per-core results back to the original query order.
"""

import math

import numpy as np

import concourse.bass as bass  # noqa: F401
import concourse.mybir as mybir
from concourse import bacc
from concourse.bass_utils import run_bass_kernel_spmd
from concourse.library_config import mlp

N_ENT = 15000
N_CORES = 8
LOGB = 19            # total buckets = 2**LOGB; 2 gather streams per core
HALF = 32768         # rows per gather stream (int16 index limit)
P = 128

LAST_RESULTS = None  # BassKernelResults of the most recent kernel() call

N_QUEUES = 4         # SWDGE queues; each runs on its own Q7 cpu pair
CHUNK_BLOCKS = 8     # 128-query blocks per dma_gather chunk


def _plan_chunks(G_half: int):
    """Chunks of (g0, cb, queue, half) covering 2*G_half blocks.

    Each chunk stays within one table half. Chunks go round-robin over
    the 4 SWDGE queues (cpu pairs run in parallel; a pair's own chunks
    are serial). A chunk's DMA-completion sem lags its descgen end by
    roughly 0.7us per block (ring drain), so chunk sizes DESCEND toward
    the end of the stream to keep the tail short.
    """
    def split4(n):
        """n split into 4 near-equal parts."""
        b = n // N_QUEUES
        r = n - b * N_QUEUES
        return [b + (1 if i < r else 0) for i in range(N_QUEUES)]

    rounds = []  # each round: list of (cb, h) per queue, cb may be 0
    if G_half >= 2 * N_QUEUES + 8:
        h0 = split4(G_half)
        h1 = split4(G_half)
        # half0: a small first round (early first sem for the vector
        # engine) + the bulk; half1: bulk + descending 3,1 tail
        rounds.append([(min(4, c - 1), 0) for c in h0])
        rounds.append([(c - min(4, c - 1), 0) for c in h0])
        rounds.append([(c - 4, 1) for c in h1])
        rounds.append([(3, 1) for _ in h1])
        rounds.append([(1, 1) for _ in h1])
    else:  # tiny problem fallback: one chunk per queue per half
        rounds.append([(c, 0) for c in split4(G_half)])
        rounds.append([(c, 1) for c in split4(G_half)])

    chunks = []
    g0 = 0
    for rnd in rounds:
        for q, (cb, h) in enumerate(rnd):
            if cb <= 0:
                continue
            chunks.append((g0, cb, q, h))
            g0 += cb
    assert g0 == 2 * G_half, (g0, G_half)
    return chunks


def _build_nc(G_half: int, CAP: int, CAPC: int, tag_dt: "mybir.dt"):
    """Device program: probe 2*G_half*128 queries against two [HALF, CAP]
    tag-table halves.

    CAP is the gathered row length (dma_gather needs 256B multiples);
    CAPC <= CAP is the occupied prefix actually compared.
    """
    nc = bacc.Bacc(
        "TRN2", target_bir_lowering=False, debug=False, num_swdge_queues=N_QUEUES
    )
    G = 2 * G_half
    Qc = G * P
    chunks = _plan_chunks(G_half)
    max_cb = max(cb for _, cb, _, _ in chunks)

    t0 = nc.dram_tensor("t0", [HALF, CAP], tag_dt, kind="ExternalInput")
    t1 = nc.dram_tensor("t1", [HALF, CAP], tag_dt, kind="ExternalInput")
    idxw_d = nc.dram_tensor("idxw", [P, Qc // 16], mybir.dt.int16, kind="ExternalInput")
    qtag_d = nc.dram_tensor("qtag", [P, G], tag_dt, kind="ExternalInput")
    out_d = nc.dram_tensor("hit", [P, G], mybir.dt.float32, kind="ExternalOutput")

    with (
        nc.Block() as block,
        nc.sbuf_tensor("iw", [P, Qc // 16], mybir.dt.int16) as iw,
        nc.sbuf_tensor("tagt", [P, G], tag_dt) as tagt,
        nc.sbuf_tensor("gt", [P, G, CAP], tag_dt) as gt,
        nc.sbuf_tensor("eq", [P, max_cb, CAPC], mybir.dt.bfloat16) as eq,
        nc.sbuf_tensor("m", [P, G], mybir.dt.bfloat16) as m,
        nc.sbuf_tensor("res", [P, G], mybir.dt.float32) as res,
        nc.semaphore("s_iw") as s_iw,
        nc.semaphore("s_tag") as s_tag,
        nc.semaphore("s_g0") as s_g0,
        nc.semaphore("s_g1") as s_g1,
        nc.semaphore("s_g2") as s_g2,
        nc.semaphore("s_g3") as s_g3,
        nc.semaphore("s_v") as s_v,
        nc.semaphore("s_out") as s_out,
    ):
        s_gs = [s_g0, s_g1, s_g2, s_g3]
        tables = [t0, t1]

        @block.gpsimd
        def _(g):
            g.load_library(mlp)
            g.wait_ge(s_iw, 16)  # idx array resident (tags only gate vector)
            for g0, cb, q, h in chunks:
                cq = cb * P
                g.dma_gather(
                    gt[:, g0 : g0 + cb, :], tables[h].ap(),
                    iw[:, g0 * (P // 16) : (g0 + cb) * (P // 16)],
                    cq, cq, CAP, single_packet=False,
                    queue_num=q,
                ).then_inc(s_gs[q], 16)

        @block.vector
        def _(v):
            seen = [0] * N_QUEUES
            for k, (g0, cb, q, h) in enumerate(chunks):
                seen[q] += 1
                v.wait_ge(s_gs[q], 16 * seen[q])
                if k == 0:
                    v.wait_ge(s_tag, 16)
                v.tensor_tensor(
                    out=eq[:, :cb, :],
                    in0=gt[:, g0 : g0 + cb, :CAPC],
                    in1=tagt[:, g0 : g0 + cb].to_broadcast([P, cb, CAPC]),
                    op=mybir.AluOpType.is_equal,
                )
                v.tensor_reduce(
                    out=m[:, g0 : g0 + cb], in_=eq[:, :cb, :],
                    axis=mybir.AxisListType.X, op=mybir.AluOpType.max,
                )
            v.tensor_scalar(
                out=res[:], in0=m[:], scalar1=10.0, scalar2=-5.0,
                op0=mybir.AluOpType.mult, op1=mybir.AluOpType.add,
            ).then_inc(s_v, 1)

        @block.sync
        def _(sy):
            sy.dma_start(iw[:], idxw_d.ap()).then_inc(s_iw, 16)
            sy.dma_start(tagt[:], qtag_d.ap()).then_inc(s_tag, 16)
            sy.wait_ge(s_v, 1)
            sy.dma_start(out_d.ap(), res[:]).then_inc(s_out, 16)
            sy.wait_ge(s_out, 16)

    nc.compile()
    return nc


def _ensure_trace_hook():
    """If BASS_TRACE is set but this image's antenv lacks axon_hooks,
    bass_utils would crash on import; synthesize the module (real ctypes
    hook when available, else a None hook so tracing degrades gracefully)."""
    import sys
    import types

    try:
        import antenv.axon_hooks  # noqa: F401
        return
    except ImportError:
        pass
    hook = None
    try:
        from trn_agent_boot.trn_boot import _ntff_profile_via_ctypes

        hook = _ntff_profile_via_ctypes("/opt/axon/libaxon_pjrt.so")
    except Exception:
        hook = None
    mod = types.ModuleType("antenv.axon_hooks")
    mod.get_axon_ntff_profile_hook = lambda: hook
    mod.set_axon_ntff_profile_hook = lambda h: None
    sys.modules["antenv.axon_hooks"] = mod


def _keys(h, r, t, int64_mode):
    """Replicates the reference's key computation."""
    if int64_mode:
        h = h.astype(np.int64)
        return (h * 15000 + r.astype(np.int64)) * 15000 + t.astype(np.int64)
    # int32 path: jax with x64 disabled wraps in int32; compute in uint32
    # (same bit pattern, well-defined wraparound).
    h = h.astype(np.uint32)
    return (h * np.uint32(15000) + r.astype(np.uint32)) * np.uint32(15000) + t.astype(
        np.uint32
    )


def kernel(heads, rels, tails, data) -> np.ndarray:
    heads = np.ascontiguousarray(heads)
    rels = np.ascontiguousarray(rels)
    tails = np.ascontiguousarray(tails)
    data = np.ascontiguousarray(data)
    Q = heads.shape[0]

    int64_mode = bool(heads.dtype == np.int64 or data.dtype == np.int64)
    keybits = 42 if int64_mode else 32
    shift = keybits - LOGB
    tag_mask = (1 << shift) - 1
    tag_np = np.int32 if shift > 15 else np.int16
    tag_dt = mybir.dt.int32 if shift > 15 else mybir.dt.int16
    # dma_gather rows must be a multiple of 256 bytes
    cap_quantum = 256 // np.dtype(tag_np).itemsize

    dk = _keys(data[0], data[1], data[2], int64_mode)
    qk = _keys(heads, rels, tails, int64_mode)

    # --- table build (host): sort keys; high bits = bucket -> contiguous runs
    B = 1 << LOGB
    nbl_core = B // N_CORES            # buckets per core (2 halves of HALF)
    assert nbl_core == 2 * HALF
    ds = np.sort(dk)
    db = (ds >> shift).astype(np.int64)
    dtag = (ds & np.array(tag_mask, dtype=ds.dtype)).astype(tag_np)
    counts = np.bincount(db, minlength=B)
    CAPC = max(8, int(math.ceil(counts.max() / 8)) * 8)          # compared slots
    CAP = max(cap_quantum, int(math.ceil(CAPC / cap_quantum)) * cap_quantum)
    starts = np.zeros(B, dtype=np.int64)
    np.cumsum(counts[:-1], out=starts[1:])
    slot = np.arange(ds.shape[0], dtype=np.int64) - starts[db]
    table = np.full((B, CAP), -1, dtype=tag_np)
    table[db, slot] = dtag

    # --- query routing (host)
    qb = (qk >> shift).astype(np.int64)
    qtag = (qk & np.array(tag_mask, dtype=qk.dtype)).astype(tag_np)
    qcore = qb >> (LOGB - 3)
    qhalf = (qb >> 15) & 1
    qlocal = (qb & (HALF - 1)).astype(np.int16)
    sels = [
        np.nonzero((qcore == c) & (qhalf == h))[0]
        for c in range(N_CORES)
        for h in (0, 1)
    ]  # index: 2*c + h
    # Sort each core-half's queries by bucket: the gather's HBM addresses
    # become ascending (~1.3KB apart), turning random 256B reads into
    # mostly row-buffer-friendly ones. The drain is HBM-latency-bound
    # (~38ns/descriptor unsorted), so this directly shortens the tail.
    sels = [s[np.argsort(qlocal[s], kind="stable")] for s in sels]
    G_half = max(1, int(math.ceil(max(len(s) for s in sels) / P)))
    G = 2 * G_half
    Qc = G * P
    chunks = _plan_chunks(G_half)

    in_maps = []
    for c in range(N_CORES):
        idx_flat = np.zeros(Qc, dtype=np.int16)      # padding gathers row 0 (harmless)
        tag_t = np.full((G, P), -2, dtype=tag_np)    # padding never matches
        for h in (0, 1):
            s = sels[2 * c + h]
            o = h * G_half * P
            idx_flat[o : o + len(s)] = qlocal[s]
            tag_t.ravel()[o : o + len(s)] = qtag[s]
            # NOTE: padding must stay 0 (gather row 0). Trimming pads with
            # -1 (the ucode drops trailing negatives) hangs the kernel on
            # hardware: the chunk's DMA-completion sem then undershoots
            # the +16 the waiters expect.
        idx_w = np.tile(idx_flat.reshape(-1, 16).T, (8, 1))  # [128, Qc//16]
        base = c * nbl_core
        in_maps.append(
            {
                "t0": table[base : base + HALF],
                "t1": table[base + HALF : base + 2 * HALF],
                "idxw": np.ascontiguousarray(idx_w),
                "qtag": np.ascontiguousarray(tag_t.T),
            }
        )

    _ensure_trace_hook()
    nc = _build_nc(G_half, CAP, CAPC, tag_dt)
    # trace_cores=all: profiling a strict subset of executing cores crashes
    # the axon NRT profile path; all-cores tracing is stable.
    r = run_bass_kernel_spmd(
        nc, in_maps, core_ids=list(range(N_CORES)),
        trace_cores=list(range(N_CORES)),
    )
    global LAST_RESULTS
    LAST_RESULTS = r

    out = np.full(Q, -5.0, dtype=np.float32)
    for c in range(N_CORES):
        res = r.results[c]["hit"]  # [P, G]
        flat = res.T.ravel()
        for h in (0, 1):
            s = sels[2 * c + h]
            o = h * G_half * P
            out[s] = flat[o : o + len(s)]
    return out
